# revision 1
# baseline (speedup 1.0000x reference)
"""Trainium2 Bass kernel for BalancedIPRMPNN (GNN message passing).

Reference computation (G=128 disjoint graphs, NPG=512 nodes each, H=128):
    h2   = x @ (W_emb @ W_gcn) + b_emb @ W_gcn          # embedding+GCN linear folded
    m    = relu(D^-1/2 (Adj + I) D^-1/2 @ h2 + b_gcn)   # GCN propagate (per graph)
    virt = einsum('gnv,gnh->gvh', edge_weights, m)      # weighted pooling (V=64)
    t1   = relu(virt @ vW1 + vb1)
    gf   = mean_v(t1 @ vW2 + vb2)
    out  = relu(gf @ mW1 + mb1) @ mW2 + mb2             # [G, 10]

Strategy: data-parallel over graphs, 16 graphs per core on 8 cores.  The
message passing runs as a dense per-graph [512,512] adjacency matmul on the
tensor engine; host folds the symmetric degree normalization into x (rows
pre-scaled by dinv) and edge_weights (rows pre-scaled by dinv), ships the
integer adjacency (exact in fp8e4) transposed for the lhsT layout.  The MLP
tail runs batched across graphs in a transposed layout so every bias is a
per-partition ScalarE activation bias.
"""

import ml_dtypes
import numpy as np

import concourse.mybir as mybir
import concourse.tile as tile
from concourse import bacc
from concourse.bass_utils import run_bass_kernel_spmd

# Problem constants (hardcoded per contract)
G, NPG, H, IN, OUT, V = 128, 512, 128, 128, 10, 64
N = G * NPG
N_CORES = 8
GPC = G // N_CORES          # graphs per core = 16
NS = GPC * NPG              # nodes per core  = 8192
KB = NPG // 128             # 4 k-blocks of 128 nodes per graph

F32 = mybir.dt.float32
F16 = mybir.dt.float16
F8 = mybir.dt.float8e4

X_FP16 = True               # ship x in fp16 (halves x DMA, 4x faster h2 matmuls)

_CACHE = {}


def _build_program(with_bias: bool):
    """Build the per-core Bass/Tile program (identical on all 8 cores)."""
    nc = bacc.Bacc("TRN2", target_bir_lowering=False)
    XDT = F16 if X_FP16 else F32

    # ---- DRAM I/O ----
    xsT = nc.dram_tensor("xsT", [IN, NS], XDT, kind="ExternalInput")          # dinv-scaled x, transposed
    W1 = nc.dram_tensor("W1", [IN, H], XDT, kind="ExternalInput")             # W_emb @ W_gcn
    # adjacency counts (+I), pre-arranged to SBUF layout, 2 graphs per row:
    # [j, p, gg*KB*NPG + kb*NPG + d]  (gg in {0,1}, graph = 2j+gg)
    adjT = nc.dram_tensor("adjT", [GPC // 2, 128, 2 * KB * NPG], F8, kind="ExternalInput")
    # dinv-scaled edge_weights, pre-arranged likewise
    ews = nc.dram_tensor("ews", [GPC // 2, 128, 2 * KB * V], F16, kind="ExternalInput")
    vW1 = nc.dram_tensor("vW1", [H, H], F32, kind="ExternalInput")
    vb1 = nc.dram_tensor("vb1", [H, 1], F32, kind="ExternalInput")
    vW2s = nc.dram_tensor("vW2s", [H, H], F32, kind="ExternalInput")          # vW2 / V
    vb2 = nc.dram_tensor("vb2", [H, 1], F32, kind="ExternalInput")
    mW1 = nc.dram_tensor("mW1", [H, H], F32, kind="ExternalInput")
    mb1 = nc.dram_tensor("mb1", [H, 1], F32, kind="ExternalInput")
    mW2 = nc.dram_tensor("mW2", [H, OUT], F32, kind="ExternalInput")
    mb2 = nc.dram_tensor("mb2", [OUT, 1], F32, kind="ExternalInput")
    if with_bias:
        biasL = nc.dram_tensor("biasL", [GPC, 2, NPG], F16, kind="ExternalInput")
        biasR = nc.dram_tensor("biasR", [2, H], F16, kind="ExternalInput")
    outT = nc.dram_tensor("outT", [OUT, GPC], F32, kind="ExternalOutput")

    with tile.TileContext(nc) as tc:
        with (
            tc.tile_pool(name="consts", bufs=1) as consts,
            tc.tile_pool(name="xchunk", bufs=4) as xchunk_pool,
            tc.tile_pool(name="upool", bufs=3) as u_pool,
            tc.tile_pool(name="adj", bufs=4) as adj_pool,
            tc.tile_pool(name="ewsp", bufs=4) as ews_pool,
            tc.tile_pool(name="mp", bufs=3) as m_pool,
            tc.tile_pool(name="blp", bufs=3) as bl_pool,
            tc.tile_pool(name="ph2", bufs=2, space="PSUM") as ph2,
            tc.tile_pool(name="pm", bufs=4, space="PSUM") as pm,
            tc.tile_pool(name="pv", bufs=1, space="PSUM") as pv,
            tc.tile_pool(name="pd", bufs=1, space="PSUM") as pd,
        ):
            # critical-path data first: graph pair 0's inputs, then W1
            xc0 = xchunk_pool.tile([IN, 2 * NPG], XDT, tag="xc")
            nc.sync.dma_start(out=xc0[:], in_=xsT[:, 0:2 * NPG])
            W1_sb = consts.tile([IN, H], XDT)
            nc.sync.dma_start(out=W1_sb[:], in_=W1[:])
            adj0 = adj_pool.tile([128, 2 * KB * NPG], F8, tag="adj")
            nc.sync.dma_start(out=adj0[:], in_=adjT[0])
            ews0 = ews_pool.tile([128, 2 * KB * V], F16, tag="ews")
            nc.sync.dma_start(out=ews0[:], in_=ews[0])

            vW1_sb = consts.tile([H, H], F32)
            nc.scalar.dma_start(out=vW1_sb[:], in_=vW1[:])
            vW2_sb = consts.tile([H, H], F32)
            nc.scalar.dma_start(out=vW2_sb[:], in_=vW2s[:])
            mW1_sb = consts.tile([H, H], F32)
            nc.scalar.dma_start(out=mW1_sb[:], in_=mW1[:])
            mW2_sb = consts.tile([H, OUT], F32)
            nc.scalar.dma_start(out=mW2_sb[:], in_=mW2[:])
            vb1_sb = consts.tile([H, 1], F32)
            nc.scalar.dma_start(out=vb1_sb[:], in_=vb1[:])
            vb2_sb = consts.tile([H, 1], F32)
            nc.scalar.dma_start(out=vb2_sb[:], in_=vb2[:])
            mb1_sb = consts.tile([H, 1], F32)
            nc.scalar.dma_start(out=mb1_sb[:], in_=mb1[:])
            mb2_sb = consts.tile([OUT, 1], F32)
            nc.scalar.dma_start(out=mb2_sb[:], in_=mb2[:])
            if with_bias:
                biasR_sb = consts.tile([2, H], F16)
                nc.scalar.dma_start(out=biasR_sb[:], in_=biasR[:])

            virtT = consts.tile([H, GPC * V], F32)  # virt^T, all graphs side by side
            t1 = consts.tile([H, GPC * V], F32)
            t1s = consts.tile([H, GPC], F32)

            def emit_embed(j):
                # u = (dinv*x) @ W1, cast fp16, for graph pair j (1024 nodes)
                if j == 0:
                    xc = xc0
                else:
                    xc = xchunk_pool.tile([IN, 2 * NPG], XDT, tag="xc")
                    nc.sync.dma_start(out=xc[:], in_=xsT[:, 2 * j * NPG:2 * (j + 1) * NPG])
                u_j = u_pool.tile([128, 2 * KB * H], F16, tag="u")
                for half in range(2):
                    p_h2 = ph2.tile([128, KB * H], F32, tag="ph2")
                    for kb in range(KB):
                        nc.tensor.matmul(
                            p_h2[:, kb * H:(kb + 1) * H],
                            xc[:, half * NPG + kb * 128: half * NPG + (kb + 1) * 128],
                            W1_sb[:],
                            start=True, stop=True,
                        )
                    nc.vector.tensor_copy(
                        out=u_j[:, half * KB * H:(half + 1) * KB * H], in_=p_h2[:])
                return u_j

            us = [emit_embed(0)]
            pending = []
            for g in range(GPC):
                j, gg = divmod(g, 2)
                if gg == 0:
                    if j + 1 < GPC // 2:
                        us.append(emit_embed(j + 1))
                    if j == 0:
                        adj_pair, ews_pair = adj0, ews0
                    else:
                        adj_pair = adj_pool.tile([128, 2 * KB * NPG], F8, tag="adj")
                        nc.sync.dma_start(out=adj_pair[:], in_=adjT[j])
                        ews_pair = ews_pool.tile([128, 2 * KB * V], F16, tag="ews")
                        nc.sync.dma_start(out=ews_pair[:], in_=ews[j])
                u_g = us[j][:, gg * KB * H:(gg + 1) * KB * H]
                adj_sb = adj_pair[:, gg * KB * NPG:(gg + 1) * KB * NPG]
                ews_sb = ews_pair[:, gg * KB * V:(gg + 1) * KB * V]
                if with_bias:
                    bl_sb = bl_pool.tile([2, NPG], F16, tag="bl")
                    nc.sync.dma_start(out=bl_sb[:], in_=biasL[g])

                m_sb = m_pool.tile([128, KB * H], F16, tag="m")
                for mb in range(KB):
                    p_m = pm.tile([128, H], F32, tag="pm")
                    if with_bias:
                        nc.tensor.matmul(
                            p_m[:], bl_sb[:, mb * 128:(mb + 1) * 128], biasR_sb[:],
                            start=True, stop=False,
                        )
                    for kb in range(KB):
                        nc.tensor.matmul(
                            p_m[:],
                            adj_sb[:, kb * NPG + mb * 128: kb * NPG + (mb + 1) * 128],
                            u_g[:, kb * H:(kb + 1) * H],
                            start=(kb == 0 and not with_bias),
                            stop=(kb == KB - 1),
                        )
                    nc.scalar.activation(
                        out=m_sb[:, mb * H:(mb + 1) * H], in_=p_m[:],
                        func=mybir.ActivationFunctionType.Relu,
                    )

                # ---- pooling (deferred by one graph so the relu is long done
                # by the time the PE reaches these matmuls) ----
                pending.append((g, m_sb, ews_sb))
                emit_g = g - 1 if g < GPC - 1 else None
                ready = [p for p in pending if p[0] == emit_g]
                if g == GPC - 1:
                    ready = list(pending)
                for eg, e_m, e_ews in ready:
                    p_v = pv.tile([128, V], F32, tag="pv")
                    for kb in range(KB):
                        nc.tensor.matmul(
                            p_v[:],
                            e_m[:, kb * H:(kb + 1) * H],
                            e_ews[:, kb * V:(kb + 1) * V],
                            start=(kb == 0), stop=(kb == KB - 1),
                        )
                    nc.vector.tensor_copy(out=virtT[:, eg * V:(eg + 1) * V], in_=p_v[:])
                    pending.remove((eg, e_m, e_ews))

                # ---- MLP first stage per quarter once its 4 graphs are emitted ----
                for q in range(4):
                    if g != (4 * q + 5 if q < 3 else GPC - 1):
                        continue
                    p_t1 = pd.tile([128, 256], F32, tag="pd")
                    nc.tensor.matmul(
                        p_t1[:], vW1_sb[:], virtT[:, q * 256:(q + 1) * 256],
                        start=True, stop=True,
                    )
                    nc.scalar.activation(
                        out=t1[:, q * 256:(q + 1) * 256], in_=p_t1[:],
                        func=mybir.ActivationFunctionType.Relu, bias=vb1_sb[:],
                    )
                    nc.vector.tensor_reduce(
                        out=t1s[:, q * 4:(q + 1) * 4],
                        in_=t1[:, q * 256:(q + 1) * 256]
                            .rearrange("p (g v) -> p g v", v=V),
                        axis=mybir.AxisListType.X, op=mybir.AluOpType.add,
                    )

            # ---- rest of the MLP tail ----
            p_gf = pd.tile([128, GPC], F32, tag="pd")
            nc.tensor.matmul(p_gf[:], vW2_sb[:], t1s[:], start=True, stop=True)
            gf = consts.tile([H, GPC], F32)
            nc.scalar.activation(
                out=gf[:], in_=p_gf[:],
                func=mybir.ActivationFunctionType.Identity, bias=vb2_sb[:],
            )
            p_q1 = pd.tile([128, GPC], F32, tag="pd")
            nc.tensor.matmul(p_q1[:], mW1_sb[:], gf[:], start=True, stop=True)
            q1 = consts.tile([H, GPC], F32)
            nc.scalar.activation(
                out=q1[:], in_=p_q1[:],
                func=mybir.ActivationFunctionType.Relu, bias=mb1_sb[:],
            )
            p_o = pd.tile([OUT, GPC], F32, tag="pd")
            nc.tensor.matmul(p_o[:], mW2_sb[:], q1[:], start=True, stop=True)
            o_sb = consts.tile([OUT, GPC], F32)
            nc.scalar.activation(
                out=o_sb[:], in_=p_o[:],
                func=mybir.ActivationFunctionType.Identity, bias=mb2_sb[:],
            )
            nc.sync.dma_start(out=outT[:], in_=o_sb[:])

    nc.finalize()
    return nc


def _reference_numpy(x, edge_index, W_emb, b_emb, W_gcn, b_gcn, edge_weights,
                     vW1, vb1, vW2, vb2, mW1, mb1, mW2, mb2):
    """Pure-numpy fallback (used only if graphs are not disjoint)."""
    src, dst = edge_index[0].astype(np.int64), edge_index[1].astype(np.int64)
    h = x @ W_emb + b_emb
    h2 = h @ W_gcn
    deg = np.bincount(dst, minlength=N).astype(np.float32) + 1.0
    dinv = 1.0 / np.sqrt(deg)
    m = np.zeros_like(h2)
    np.add.at(m, dst, h2[src] * (dinv[src] * dinv[dst])[:, None])
    m += h2 * (dinv * dinv)[:, None]
    m = np.maximum(m + b_gcn, 0.0)
    hg = m.reshape(G, NPG, -1)
    virt = np.einsum('gnv,gnh->gvh', edge_weights, hg)
    t1 = np.maximum(virt @ vW1 + vb1, 0.0) @ vW2 + vb2
    gf = t1.mean(axis=1)
    return np.maximum(gf @ mW1 + mb1, 0.0) @ mW2 + mb2


def kernel(x, edge_index, batch, W_emb, b_emb, W_gcn, b_gcn, edge_weights,
           vW1, vb1, vW2, vb2, mW1, mb1, mW2, mb2, _trace=False):
    x = np.asarray(x, dtype=np.float32)
    edge_index = np.asarray(edge_index, dtype=np.int32)
    W_emb = np.asarray(W_emb, dtype=np.float32)
    b_emb = np.asarray(b_emb, dtype=np.float32)
    W_gcn = np.asarray(W_gcn, dtype=np.float32)
    b_gcn = np.asarray(b_gcn, dtype=np.float32)
    edge_weights = np.asarray(edge_weights, dtype=np.float32)
    vW1, vb1 = np.asarray(vW1, np.float32), np.asarray(vb1, np.float32)
    vW2, vb2 = np.asarray(vW2, np.float32), np.asarray(vb2, np.float32)
    mW1, mb1 = np.asarray(mW1, np.float32), np.asarray(mb1, np.float32)
    mW2, mb2 = np.asarray(mW2, np.float32), np.asarray(mb2, np.float32)

    src = edge_index[0].astype(np.int64)
    dst = edge_index[1].astype(np.int64)
    if not np.array_equal(src // NPG, dst // NPG):
        # cross-graph edges: dense per-graph adjacency doesn't apply
        return _reference_numpy(x, edge_index, W_emb, b_emb, W_gcn, b_gcn,
                                edge_weights, vW1, vb1, vW2, vb2, mW1, mb1,
                                mW2, mb2).astype(np.float32)

    # ---- host prep ----
    deg = (np.bincount(dst, minlength=N) + 1).astype(np.float32)  # in-degree + self loop
    dinv = (1.0 / np.sqrt(deg)).astype(np.float32)

    # per-graph transposed adjacency counts (+ self loops), exact small ints in fp8e4
    gidx = src // NPG
    lin = (gidx * NPG + (src % NPG)) * NPG + (dst % NPG)
    counts = np.bincount(lin, minlength=G * NPG * NPG)
    adjT_all = counts.reshape(G, NPG, NPG).astype(np.float32)
    diag = np.arange(NPG)
    adjT_all[:, diag, diag] += np.float32(1.0)
    if adjT_all.max() > 16:  # not exactly representable in fp8e4
        return _reference_numpy(x, edge_index, W_emb, b_emb, W_gcn, b_gcn,
                                edge_weights, vW1, vb1, vW2, vb2, mW1, mb1,
                                mW2, mb2).astype(np.float32)
    adjT_all = adjT_all.astype(ml_dtypes.float8_e4m3)
    # SBUF layout: [g, p, kb*NPG + d], then merge graph pairs so each DMA is
    # one [128, contiguous] block covering 2 graphs
    adjT_sb_all = (
        adjT_all.reshape(G, KB, 128, NPG).transpose(0, 2, 1, 3).reshape(G, 128, KB * NPG)
    )
    adjT_sb_all = np.ascontiguousarray(
        adjT_sb_all.reshape(G // 2, 2, 128, KB * NPG).transpose(0, 2, 1, 3)
        .reshape(G // 2, 128, 2 * KB * NPG)
    )

    xdt = np.float16 if X_FP16 else np.float32
    xs = (x * dinv[:, None])  # fold D^-1/2 into x rows
    xsT_np = np.ascontiguousarray(xs.T.astype(xdt))  # [IN, N]
    ews_all = (edge_weights * dinv.reshape(G, NPG, 1)).astype(np.float16)
    ews_sb_all = (
        ews_all.reshape(G, KB, 128, V).transpose(0, 2, 1, 3).reshape(G, 128, KB * V)
    )
    ews_sb_all = np.ascontiguousarray(
        ews_sb_all.reshape(G // 2, 2, 128, KB * V).transpose(0, 2, 1, 3)
        .reshape(G // 2, 128, 2 * KB * V)
    )

    W1h = (W_emb @ W_gcn).astype(xdt)
    vW2s_h = (vW2 / np.float32(V)).astype(np.float32)
    bvec = (b_emb @ W_gcn).astype(np.float32)
    with_bias = bool(np.any(bvec) or np.any(b_gcn))
    if with_bias:
        # m-psum bias = wvec ⊗ bvec + sqrt(deg) ⊗ b_gcn, with
        # wvec = (Adj+I) @ dinv per graph (host-computable rank-2 correction)
        dinv_g = dinv.reshape(G, NPG)
        wvec = np.einsum('gsd,gs->gd', adjT_all.astype(np.float32), dinv_g)
        sdeg = np.sqrt(deg).reshape(G, NPG)
        biasL_all = np.stack([wvec, sdeg], axis=1).astype(np.float16)  # [G, 2, NPG]
        biasR_np = np.stack([bvec, b_gcn], axis=0).astype(np.float16)  # [2, H]

    key = with_bias
    if key not in _CACHE:
        _CACHE[key] = _build_program(with_bias)
    nc = _CACHE[key]

    in_maps = []
    for c in range(N_CORES):
        gs = slice(c * GPC, (c + 1) * GPC)
        ps = slice(c * GPC // 2, (c + 1) * GPC // 2)
        im = {
            "xsT": np.ascontiguousarray(xsT_np[:, c * NS:(c + 1) * NS]),
            "W1": W1h,
            "adjT": adjT_sb_all[ps],
            "ews": ews_sb_all[ps],
            "vW1": vW1, "vb1": vb1.reshape(H, 1),
            "vW2s": vW2s_h, "vb2": vb2.reshape(H, 1),
            "mW1": mW1, "mb1": mb1.reshape(H, 1),
            "mW2": mW2, "mb2": mb2.reshape(OUT, 1),
        }
        if with_bias:
            im["biasL"] = np.ascontiguousarray(biasL_all[gs])
            im["biasR"] = biasR_np
        in_maps.append(im)

    res = run_bass_kernel_spmd(
        nc, in_maps, core_ids=list(range(N_CORES)), trace=_trace,
    )
    out = np.concatenate([res.results[c]["outT"].T for c in range(N_CORES)], axis=0)
    if _trace:
        kernel.last_exec_time_ns = res.exec_time_ns
        kernel.last_results = res
    return out.astype(np.float32)



# revision 59
# speedup vs baseline: 1.6076x; 1.6076x over previous
"""Trainium2 Bass kernel for BalancedIPRMPNN (GNN message passing).

Reference computation (G=128 disjoint graphs, NPG=512 nodes each, H=128):
    h    = x @ W_emb + b_emb
    m    = relu(GCN(h))                                  # sym-norm propagate
    virt = einsum('gnv,gnh->gvh', edge_weights, m)       # pooling (V=64)
    t1   = relu(virt @ vW1 + vb1) @ vW2 + vb2
    gf   = mean_v(t1)
    out  = relu(gf @ mW1 + mb1) @ mW2 + mb2              # [G, 10]

Key structural facts exploited (checked at runtime, numpy fallback if absent):
  * graphs are disjoint -> dense per-graph [512,512] adjacency matmul
  * edge_weights is v-uniform and nonnegative (reference uses ones/V), so all
    V virtual nodes are identical and pooling collapses to a weighted row-sum;
    the whole virtual-node MLP + mean + final MLP runs on one [H, G] tile,
    with gf's linear pair folded on the host (W23 = vW2 @ mW1).

Per graph the device does 3 tensor-engine matmuls + 3 cheap fused ops:
    P    = x_hat^T @ C            (2 fp8 DoubleRow matmuls, contraction 512)
    P_sb = P * colw_bcast         (DVE multiply-cast fp16; colw = dinv*ew0*V
                                   broadcast across partitions on Pool)
    M    = W1^T @ P_sb            (W1 = W_emb @ W_gcn fp16)
    s_g  = rowsum(relu(M))        (scalar activation accum_out, in-place)
where x_hat = x * dinv_src (fp8) and C = exact integer counts + I (fp8).

Sharding: data-parallel over graphs, 16 graphs per core on 8 cores.
"""

import ml_dtypes
import numpy as np

import concourse.mybir as mybir
import concourse.tile as tile
from concourse import bacc
from concourse.bass_utils import run_bass_kernel_spmd

# Problem constants (hardcoded per contract)
G, NPG, H, IN, OUT, V = 128, 512, 128, 128, 10, 64
N = G * NPG
N_CORES = 8
GPC = G // N_CORES          # graphs per core = 16
KB = NPG // 128             # 4 k-blocks of 128 nodes per graph
CHG = 8                     # graphs per x-chunk DMA
NCH = GPC // CHG            # x chunks per core = 2
CWG = 4                     # graphs per colw broadcast chunk

F32 = mybir.dt.float32
F16 = mybir.dt.float16
F8 = mybir.dt.float8e4

# packed tail-weight tile columns: vW1/V | W23 | mW2 | vb1 b23 mb2
TW_COLS = 2 * H + OUT + 3
C_VW1, C_W23, C_MW2 = 0, H, 2 * H
C_VB1, C_B23, C_MB2 = 2 * H + OUT, 2 * H + OUT + 1, 2 * H + OUT + 2

FOLD_COLW = True     # fold colw into the fp8 adjacency (drops the Pool
                     # broadcast + DVE multiply stage; costs ~0.3% extra err)

_CACHE = {}
_last_nc = None


def _build_program(with_bias: bool, variant=0):
    """Build the per-core Bass/Tile program (identical on all 8 cores)."""
    nc = bacc.Bacc("TRN2", target_bir_lowering=False)

    # ---- DRAM I/O ----
    # x_hat, fp8, SBUF layout, 8 graphs per chunk: [c, p, (g kb) * IN]
    xch = nc.dram_tensor("xch", [NCH, 128, CHG * KB * IN], F8, kind="ExternalInput")
    # adjacency counts + I (exact small ints), 2 graphs per row: [j, p, (gg kb) * NPG]
    adjp = nc.dram_tensor("adjp", [GPC // 2, 128, 2 * KB * NPG], F8, kind="ExternalInput")
    W1 = nc.dram_tensor("W1", [IN, H], F16, kind="ExternalInput")
    TW = nc.dram_tensor("TW", [128, TW_COLS], F32, kind="ExternalInput")
    if not FOLD_COLW:
        CW = nc.dram_tensor("CW", [1, GPC * NPG], F16, kind="ExternalInput")
    if with_bias:
        biasL = nc.dram_tensor("biasL", [GPC, 2, NPG], F16, kind="ExternalInput")
        biasR = nc.dram_tensor("biasR", [2, H], F16, kind="ExternalInput")
    outT = nc.dram_tensor("outT", [OUT, GPC], F32, kind="ExternalOutput")

    DR = mybir.MatmulPerfMode.DoubleRow
    Relu = mybir.ActivationFunctionType.Relu

    with tile.TileContext(nc) as tc:
        with (
            tc.tile_pool(name="consts", bufs=1) as consts,
            tc.tile_pool(name="xp", bufs=3) as xpool,
            tc.tile_pool(name="adj", bufs=6) as apool,
            tc.tile_pool(name="psb", bufs=4) as psb_pool,
            tc.tile_pool(name="cwp", bufs=3) as cw_pool,
            tc.tile_pool(name="blp", bufs=3) as bl_pool,
            tc.tile_pool(name="pP", bufs=3, space="PSUM") as pP,
            tc.tile_pool(name="pM", bufs=3, space="PSUM") as pM,
            tc.tile_pool(name="pT", bufs=2, space="PSUM") as pT,
        ):
            # ---- input DMAs: all on the SP queue so the HWDGE issue order is
            # exactly program order.  One tile per DMA: the tile framework
            # serializes readers behind every prior writer of a tile.
            x01 = xpool.tile([128, 2 * KB * IN], F8)        # graphs 0-1
            x27 = xpool.tile([128, 6 * KB * IN], F8)        # graphs 2-7
            x8f = xpool.tile([128, CHG * KB * IN], F8)      # graphs 8-15
            adj_tiles = {}

            def dma_adj(g):
                t = apool.tile([128, KB * NPG], F8, tag="a")
                j, gg = divmod(g, 2)
                nc.sync.dma_start(
                    out=t[:], in_=adjp[j, :, gg * KB * NPG:(gg + 1) * KB * NPG])
                adj_tiles[g] = t

            nc.sync.dma_start(out=x01[:], in_=xch[0, :, 0:2 * KB * IN])
            dma_adj(0)
            if not FOLD_COLW:
                CW_sb = consts.tile([1, GPC * NPG], F16)
                nc.sync.dma_start(out=CW_sb[:], in_=CW[:])
            W1_sb = consts.tile([IN, H], F16)
            nc.sync.dma_start(out=W1_sb[:], in_=W1[:])
            dma_adj(1)
            nc.sync.dma_start(out=x27[:], in_=xch[0, :, 2 * KB * IN:])
            dma_adj(2)
            dma_adj(3)
            TW_sb = consts.tile([128, TW_COLS], F32)
            nc.sync.dma_start(out=TW_sb[:], in_=TW[:])
            dma_adj(4)
            dma_adj(5)
            if with_bias:
                biasR_sb = consts.tile([2, H], F16)
                nc.sync.dma_start(out=biasR_sb[:], in_=biasR[:])

            def x_slice(g):
                if g < 2:
                    return x01, g
                if g < 8:
                    return x27, g - 2
                return x8f, g - 8

            # preload the Relu activation table while input DMAs stream
            warm = consts.tile([128, 1], F32)
            nc.gpsimd.memset(warm[:], 0.0)
            warm2 = consts.tile([128, 1], F32)
            nc.scalar.activation(out=warm2[:], in_=warm[:], func=Relu)

            s_all = consts.tile([H, GPC], F32)   # V * virt^T, one col per graph

            # colw broadcasts: all resident, issued up front on Pool
            cw_tiles = {}
            if not FOLD_COLW:
                for j in range(GPC // 2):
                    cwt = consts.tile([128, 2 * NPG], F16, name=f"cwp{j}")
                    nc.gpsimd.partition_broadcast(
                        cwt[:], CW_sb[0:1, 2 * j * NPG:(2 * j + 2) * NPG])
                    cw_tiles[j] = cwt

            # software-pipelined loop: stage A(g) = DR matmuls + colw mult;
            # stage B(g) = W1 matmul + fused relu/row-sum.
            stash = {}

            def stage_a(g):
                xt, gc = x_slice(g)
                at = adj_tiles[g]
                P_ps = pP.tile([128, NPG], F32, tag="P")
                for t in (0, 1):
                    lo = (gc * KB + 2 * t) * IN
                    lhsT = xt[:, lo:lo + 2 * IN].rearrange("p (two c) -> p two c", two=2)
                    if isinstance(at, tuple):
                        rhs = at[t][:].rearrange("p (two d) -> p two d", two=2)
                    else:
                        ro = 2 * t * NPG
                        rhs = at[:, ro:ro + 2 * NPG].rearrange("p (two d) -> p two d", two=2)
                    nc.tensor.matmul(P_ps[:], lhsT, rhs,
                                     start=(t == 0), stop=(t == 1), perf_mode=DR)
                # P_sb: cast fp16; when colw is not folded into the
                # adjacency, multiply by the broadcast colw row here
                P_sb = psb_pool.tile([128, NPG], F16, tag="psb")
                if FOLD_COLW:
                    nc.vector.tensor_copy(out=P_sb[:], in_=P_ps[:])
                else:
                    cwt = cw_tiles[g // 2]
                    nc.vector.tensor_tensor(
                        out=P_sb[:], in0=P_ps[:],
                        in1=cwt[:, (g % 2) * NPG:(g % 2 + 1) * NPG],
                        op=mybir.AluOpType.mult)
                stash[g] = P_sb

            def stage_b(g):
                P_sb = stash.pop(g)
                M_ps = pM.tile([128, NPG], F32, tag="M")
                if with_bias:
                    bl = bl_pool.tile([2, NPG], F16, tag="bl")
                    nc.scalar.dma_start(out=bl[:], in_=biasL[g])
                    nc.tensor.matmul(M_ps[:], biasR_sb[:], bl[:], start=True, stop=False)
                nc.tensor.matmul(M_ps[:], W1_sb[:], P_sb[:],
                                 start=not with_bias, stop=True)
                # fused relu + row-sum; the relu'd values are only needed for
                # the sum, so write them back in place
                nc.scalar.activation(out=M_ps[:], in_=M_ps[:], func=Relu,
                                     accum_out=s_all[:, g:g + 1])

            def prefetch(g):
                # program order is after the readers of reused buffers
                if g == 0:
                    dma_adj(6)
                elif g == 1:
                    nc.sync.dma_start(out=x8f[:], in_=xch[1])
                    dma_adj(7)
                elif 2 <= g <= 8:
                    dma_adj(g + 6)
                elif g == 9:
                    # last graph: two k-half tiles so its first DoubleRow
                    # matmul starts one half-transfer earlier
                    j = GPC // 2 - 1
                    ta = apool.tile([128, 2 * NPG], F8, name="a15a")
                    nc.sync.dma_start(
                        out=ta[:], in_=adjp[j, :, KB * NPG:KB * NPG + 2 * NPG])
                    tb = apool.tile([128, 2 * NPG], F8, name="a15b")
                    nc.sync.dma_start(
                        out=tb[:], in_=adjp[j, :, KB * NPG + 2 * NPG:])
                    adj_tiles[15] = (ta, tb)

            # MLP tail, computed per 4-graph chunk as the s columns land
            t1 = consts.tile([H, GPC], F32)
            q1 = consts.tile([H, GPC], F32)
            o_sb = consts.tile([OUT, GPC], F32)
            mx = mybir.AluOpType.max
            add = mybir.AluOpType.add
            byp = mybir.AluOpType.bypass

            def tail_chunk(lo, hi):
                cs = slice(lo, hi)
                w = hi - lo
                pt1 = pT.tile([128, w], F32, tag="t", name="pt1")
                nc.tensor.matmul(pt1[:], TW_sb[:, C_VW1:C_VW1 + H], s_all[:, cs],
                                 start=True, stop=True)
                nc.vector.tensor_scalar(out=t1[:, cs], in0=pt1[:],
                                        scalar1=TW_sb[:, C_VB1:C_VB1 + 1],
                                        scalar2=0.0, op0=add, op1=mx)
                pt2 = pT.tile([128, w], F32, tag="t", name="pt2")
                nc.tensor.matmul(pt2[:], TW_sb[:, C_W23:C_W23 + H], t1[:, cs],
                                 start=True, stop=True)
                nc.vector.tensor_scalar(out=q1[:, cs], in0=pt2[:],
                                        scalar1=TW_sb[:, C_B23:C_B23 + 1],
                                        scalar2=0.0, op0=add, op1=mx)
                pt3 = pT.tile([OUT, w], F32, tag="t", name="pt3")
                nc.tensor.matmul(pt3[:], TW_sb[:, C_MW2:C_MW2 + OUT], q1[:, cs],
                                 start=True, stop=True)
                nc.vector.tensor_scalar(out=o_sb[:, cs], in0=pt3[:],
                                        scalar1=TW_sb[0:OUT, C_MB2:C_MB2 + 1],
                                        scalar2=0.0, op0=add, op1=byp)

            stage_a(0)
            for g in range(GPC):
                if g + 1 < GPC:
                    stage_a(g + 1)
                stage_b(g)
                prefetch(g)
                if g % 4 == 3:
                    tail_chunk(g - 3, g + 1)

            nc.sync.dma_start(out=outT[:], in_=o_sb[:])

    nc.finalize()
    return nc


def _reference_numpy(x, edge_index, W_emb, b_emb, W_gcn, b_gcn, edge_weights,
                     vW1, vb1, vW2, vb2, mW1, mb1, mW2, mb2):
    """Pure-numpy fallback (used only if structural assumptions fail)."""
    src, dst = edge_index[0].astype(np.int64), edge_index[1].astype(np.int64)
    h = x @ W_emb + b_emb
    h2 = h @ W_gcn
    deg = np.bincount(dst, minlength=N).astype(np.float32) + 1.0
    dinv = 1.0 / np.sqrt(deg)
    m = np.zeros_like(h2)
    np.add.at(m, dst, h2[src] * (dinv[src] * dinv[dst])[:, None])
    m += h2 * (dinv * dinv)[:, None]
    m = np.maximum(m + b_gcn, 0.0)
    hg = m.reshape(G, NPG, -1)
    virt = np.einsum('gnv,gnh->gvh', edge_weights, hg)
    t1 = np.maximum(virt @ vW1 + vb1, 0.0) @ vW2 + vb2
    gf = t1.mean(axis=1)
    return np.maximum(gf @ mW1 + mb1, 0.0) @ mW2 + mb2


def kernel(x, edge_index, batch, W_emb, b_emb, W_gcn, b_gcn, edge_weights,
           vW1, vb1, vW2, vb2, mW1, mb1, mW2, mb2):
    global _last_nc
    x = np.asarray(x, dtype=np.float32)
    edge_index = np.asarray(edge_index, dtype=np.int32)
    W_emb = np.asarray(W_emb, dtype=np.float32)
    b_emb = np.asarray(b_emb, dtype=np.float32)
    W_gcn = np.asarray(W_gcn, dtype=np.float32)
    b_gcn = np.asarray(b_gcn, dtype=np.float32)
    edge_weights = np.asarray(edge_weights, dtype=np.float32)
    vW1, vb1 = np.asarray(vW1, np.float32), np.asarray(vb1, np.float32)
    vW2, vb2 = np.asarray(vW2, np.float32), np.asarray(vb2, np.float32)
    mW1, mb1 = np.asarray(mW1, np.float32), np.asarray(mb1, np.float32)
    mW2, mb2 = np.asarray(mW2, np.float32), np.asarray(mb2, np.float32)

    def fallback():
        return _reference_numpy(x, edge_index, W_emb, b_emb, W_gcn, b_gcn,
                                edge_weights, vW1, vb1, vW2, vb2, mW1, mb1,
                                mW2, mb2).astype(np.float32)

    src = edge_index[0].astype(np.int64)
    dst = edge_index[1].astype(np.int64)
    if not np.array_equal(src // NPG, dst // NPG):
        return fallback()  # cross-graph edges: dense per-graph adj doesn't apply

    # pooling collapse requires v-uniform, nonnegative edge weights
    ew0 = edge_weights[:, :, 0]
    if not np.all(edge_weights == ew0[:, :, None]) or np.any(ew0 < 0):
        return fallback()

    # ---- host prep ----
    deg = (np.bincount(dst, minlength=N) + 1).astype(np.float32)
    dinv = (1.0 / np.sqrt(deg)).astype(np.float32)
    colw = (dinv * ew0.reshape(N) * np.float32(V)).astype(np.float32)  # per-dst

    # per-graph adjacency counts (+ self loops), exact small ints in fp8
    gidx = src // NPG
    lin = (gidx * NPG + (src % NPG)) * NPG + (dst % NPG)
    counts = np.bincount(lin, minlength=G * NPG * NPG)
    adjc = counts.reshape(G, NPG, NPG).astype(np.float32)  # [g, src, dst]
    diag = np.arange(NPG)
    adjc[:, diag, diag] += np.float32(1.0)
    if adjc.max() > 16.0 or np.abs(x).max() > 400.0 or colw.max() > 60000.0:
        return fallback()  # outside exact-fp8 / fp16 range
    if FOLD_COLW:
        adj_f = adjc * colw.reshape(G, 1, NPG)
        if adj_f.max() > 400.0:
            return fallback()
        adj8 = adj_f.astype(ml_dtypes.float8_e4m3)
    else:
        adj8 = adjc.astype(ml_dtypes.float8_e4m3)
    # per-graph SBUF layouts [g, p, kb*NPG + d] and [g, p, kb*IN + c]
    # SBUF layout [g, p, kb*NPG + d], then merge graph pairs
    adj_sb = (adj8.reshape(G, KB, 128, NPG).transpose(0, 2, 1, 3)
              .reshape(G, 128, KB * NPG))
    adj_sb = np.ascontiguousarray(
        adj_sb.reshape(G // 2, 2, 128, KB * NPG).transpose(0, 2, 1, 3)
        .reshape(G // 2, 128, 2 * KB * NPG))
    # x_hat = dinv * x, fp8, SBUF layout [g, p, kb*IN + c], merged CHG graphs
    xs8 = (x * dinv[:, None]).astype(ml_dtypes.float8_e4m3)
    x_sb = (xs8.reshape(G, KB, 128, IN).transpose(0, 2, 1, 3)
            .reshape(G, 128, KB * IN))
    x_sb = np.ascontiguousarray(
        x_sb.reshape(G // CHG, CHG, 128, KB * IN).transpose(0, 2, 1, 3)
        .reshape(G // CHG, 128, CHG * KB * IN))

    W1h = (W_emb @ W_gcn).astype(np.float16)
    W23 = (vW2 @ mW1).astype(np.float32)
    b23 = (mW1.T @ vb2 + mb1).astype(np.float32)
    TW_np = np.zeros((128, TW_COLS), np.float32)
    TW_np[:, C_VW1:C_VW1 + H] = vW1 / np.float32(V)
    TW_np[:, C_W23:C_W23 + H] = W23
    TW_np[:, C_MW2:C_MW2 + OUT] = mW2
    TW_np[:, C_VB1] = vb1
    TW_np[:, C_B23] = b23
    TW_np[:OUT, C_MB2] = mb2

    colw_g = colw.reshape(G, NPG)

    bvec = (b_emb @ W_gcn).astype(np.float32)
    with_bias = bool(np.any(bvec) or np.any(b_gcn))
    if with_bias:
        # pre-relu rank-2 correction: bvec (x) colw*wvec0 + b_gcn (x) V*ew0
        dinv_g = dinv.reshape(G, NPG)
        wvec0 = np.einsum('gsd,gs->gd', adjc, dinv_g)           # (A+I)^T dinv
        bL0 = colw_g * wvec0
        bL1 = np.float32(V) * ew0
        biasL_all = np.stack([bL0, bL1], axis=1).astype(np.float16)  # [G, 2, NPG]
        biasR_np = np.stack([bvec, b_gcn], axis=0).astype(np.float16)

    key = with_bias
    if key not in _CACHE:
        _CACHE[key] = _build_program(with_bias)
    nc = _CACHE[key]
    _last_nc = nc

    in_maps = []
    for c in range(N_CORES):
        im = {
            "xch": x_sb[c * NCH:(c + 1) * NCH],
            "adjp": adj_sb[c * GPC // 2:(c + 1) * GPC // 2],
            "W1": W1h,
            "TW": TW_np,
        }
        if not FOLD_COLW:
            im["CW"] = np.ascontiguousarray(
                colw_g[c * GPC:(c + 1) * GPC].reshape(1, GPC * NPG)
            ).astype(np.float16)
        if with_bias:
            im["biasL"] = np.ascontiguousarray(biasL_all[c * GPC:(c + 1) * GPC])
            im["biasR"] = biasR_np
        in_maps.append(im)

    res = run_bass_kernel_spmd(nc, in_maps, core_ids=list(range(N_CORES)))
    out = np.concatenate([res.results[c]["outT"].T for c in range(N_CORES)], axis=0)
    kernel.last_results = res
    return out.astype(np.float32)
